# revision 1
# baseline (speedup 1.0000x reference)
"""Trainium2 Bass kernel for nn_CustomAttentionLayer (GQA attention + RoPE + o_proj).

Sharding: 8-way over (batch, query-chunk): core c handles batch c//4, query rows
[(c%4)*512, (c%4)*512+512). Each core computes full K/V for its batch (4x
redundant k/v projection, zero collectives), attention for all 16 heads on its
512 query rows, and the output projection for those rows.

Device layout notes:
- All DRAM inputs are pre-transposed on the host so every DMA is contiguous:
  hT = hidden[b].T with the core's query chunk rotated to columns 0..511,
  wqT/wkT = W.T with rope-permuted columns (even dims then odd dims per head),
  wvT = Wv.T, woT = Wo.T, cos/sin tables transposed and column-rotated the same
  way as hT.
- All matmuls run as float32r (HW rounds inputs to 11-bit mantissa, fp32
  accumulate in PSUM; ~1.6e-4 rel err per matmul, measured on HW).
- Softmax skips the max-subtraction (scores are O(+-5) so exp is safe in fp32
  and the result is mathematically identical); row sums come from a ones-vector
  matmul on the transposed probs; 1/rowsum is broadcast across partitions with
  a K=1 matmul and folded into the attention-output evacuation.
"""

import os
import numpy as np

import concourse.bass as bass
import concourse.mybir as mybir
import concourse.tile as tile
from concourse import bacc
from concourse.bass_utils import run_bass_kernel_spmd

B, S, H = 2, 2048, 2048
NH, NKV, HD = 16, 4, 128
SQ = 512                      # query rows per core
NC = 8                        # cores
KT = H // 128                 # 16 contraction tiles over H
SJ = S // 128                 # 16 key-position tiles
SCALE = 1.0 / float(np.sqrt(HD))

f32 = mybir.dt.float32
f32r = mybir.dt.float32r
FP = mybir.ActivationFunctionType
ALU = mybir.AluOpType


def _body(nc, tc, t):
    """Emit the whole per-core program. `t` maps tensor names -> DRAM APs."""
    hT, wqT, wkT, wvT, woT, ccD, ssD, outD = (
        t["hT"], t["wqT"], t["wkT"], t["wvT"], t["woT"], t["cc"], t["ss"],
        t["out"]
    )
    with tc.tile_pool(name="main", bufs=1) as main, tc.tile_pool(
        name="psum", bufs=1, space="PSUM"
    ) as pp:
        # cc = [cos; cos], ss = [sin; sin] stacked on the partition dim
        # (DMA emission deferred so the first wq/hq loads go out first)
        cc = main.tile([128, S], f32, tag="cc", bufs=1)
        ss = main.tile([128, S], f32, tag="ss", bufs=1)
        qts = [main.tile([128, SQ], f32r, tag="qt", bufs=NH, name=f"qt{i}")
               for i in range(NH)]
        kts = [main.tile([128, S], f32r, tag="kt", bufs=NKV, name=f"kt{i}")
               for i in range(NKV)]
        vts = [main.tile([128, NKV * HD], f32r, tag="v", bufs=SJ, name=f"v{i}")
               for i in range(SJ)]

        def rope(dst, ps, cols, pool):
            # With x = [xr; xi] on partition halves and cc = [c; c], ss = [s; s]:
            #   A = x*cc = (xr*c | xi*c),  B = x*ss = (xr*s | xi*s)
            #   dst[0:64]   = xr*c - xi*s = A[0:64]  - B[64:128]
            #   dst[64:128] = xr*s + xi*c = B[0:64]  + A[64:128]
            w = dst.shape[-1]
            t1 = pool.tile([64, w], f32, tag="ropeA", bufs=2, name="t1")
            t2 = pool.tile([64, w], f32, tag="ropeB", bufs=1, name="t2")
            nc.vector.tensor_tensor(t1[:], ps[64:128, :], ss[64:128, cols],
                                    op=ALU.mult)
            nc.vector.tensor_tensor(t2[:], ps[0:64, :], cc[0:64, cols],
                                    op=ALU.mult)
            nc.vector.tensor_sub(dst[0:64, :], t2[:], t1[:])
            nc.vector.tensor_tensor(t1[:], ps[0:64, :], ss[0:64, cols],
                                    op=ALU.mult)
            nc.vector.tensor_tensor(t2[:], ps[64:128, :], cc[64:128, cols],
                                    op=ALU.mult)
            nc.vector.tensor_add(dst[64:128, :], t2[:], t1[:])

        # ---------------- projections (streamed over 4 column-quarters) ----
        # Weight DRAM views packed so one DMA loads all 16 contraction tiles
        # of a 128-wide output block: [H, n] -> [p, k, n-slice].
        wqT_p = wqT.rearrange("(k p) n -> p k n", p=128)
        wkT_p = wkT.rearrange("(k p) n -> p k n", p=128)
        wvT_p = wvT.rearrange("(k p) n -> p k n", p=128)
        with tc.tile_pool(name="projtmp", bufs=1) as pt:
            wkr = []

            def load_wk():
                for m in range(NKV):
                    wkm = pt.tile([128, KT * 128], f32r, tag="wkr", bufs=NKV,
                                  name=f"wkr{m}")
                    nc.sync.dma_start(
                        wkm[:].rearrange("p (k j) -> p k j", j=128),
                        wkT_p[:, :, bass.ts(m, 128)],
                    )
                    wkr.append(wkm)

            wqm_tiles = {}

            def prefetch_wq(m):
                if m < NH:
                    wqm = pt.tile([128, KT * 128], f32r, tag="wqm", bufs=2,
                                  name="wqm")
                    nc.sync.dma_start(
                        wqm[:].rearrange("p (k j) -> p k j", j=128),
                        wqT_p[:, :, bass.ts(m, 128)],
                    )
                    wqm_tiles[m] = wqm

            prefetch_wq(0)

            for qt in range(4):
                cols = bass.ts(qt, 512)
                hq = []
                for k in range(KT):
                    hk = pt.tile([128, 512], f32r, tag="hq", bufs=16,
                                 name=f"hq{qt}_{k}")
                    nc.sync.dma_start(hk[:], hT[bass.ts(k, 128), cols])
                    hq.append(hk)

                # rope tables for this quarter's columns only (keeps the
                # startup DMA queue clear for the first weight blocks)
                if qt == 0:
                    prefetch_wq(1)
                nc.sync.dma_start(cc[:, cols], ccD[:, cols])
                nc.sync.dma_start(ss[:, cols], ssD[:, cols])

                if qt == 0:
                    # q projection for the core's chunk (= quarter 0)
                    for m in range(NH):
                        wqm = wqm_tiles.pop(m)
                        prefetch_wq(m + 2)
                        ps = pp.tile([128, 512], f32, tag="mm", bufs=4, name="psq")
                        for k in range(KT):
                            nc.tensor.matmul(
                                ps[:], wqm[:, bass.ts(k, 128)], hq[k][:],
                                start=(k == 0), stop=(k == KT - 1),
                            )
                        rope(qts[m], ps, slice(0, SQ), pt)
                    load_wk()

                # k projection: 4 kv heads, this quarter's key positions
                for m in range(NKV):
                    ps = pp.tile([128, 512], f32, tag="mm", bufs=4, name="psk")
                    for k in range(KT):
                        nc.tensor.matmul(
                            ps[:], wkr[m][:, bass.ts(k, 128)], hq[k][:],
                            start=(k == 0), stop=(k == KT - 1),
                        )
                    rope(kts[m][:, cols], ps, cols, pt)

                # v projection for this quarter's rows; Wv streamed per k-tile
                pvs = [pp.tile([128, 512], f32, tag="acc4", bufs=4,
                               name=f"psv{i}") for i in range(4)]
                for k in range(KT):
                    wv_k = pt.tile([128, 512], f32r, tag="wv", bufs=4,
                                   name="wvk")
                    nc.sync.dma_start(wv_k[:], wvT[bass.ts(k, 128), :])
                    for sub in range(4):
                        nc.tensor.matmul(
                            pvs[sub][:], hq[k][:, bass.ts(sub, 128)], wv_k[:],
                            start=(k == 0), stop=(k == KT - 1),
                        )
                for sub in range(4):
                    nc.scalar.copy(vts[qt * 4 + sub][:, :], pvs[sub][:])

        # ---------------- attention ----------------------------------------
        with tc.tile_pool(name="attn", bufs=1) as at:
            ones_f = at.tile([128, 128], f32, tag="ones_f", bufs=1)
            nc.vector.memset(ones_f[:], 1.0)
            ones_sq = at.tile([128, 128], f32r, tag="ones_sq", bufs=1)
            nc.vector.tensor_copy(ones_sq[:], ones_f[:])

            onorm = [at.tile([128, SQ], f32r, tag="onorm", bufs=NH,
                             name=f"on{h}") for h in range(NH)]
            for h in range(NH):
                g = h // (NH // NKV)
                pv = pp.tile([128, 512], f32, tag="acc4", bufs=4, name="pspv")
                # rowsum broadcast to all 128 partitions: ones[128,128].T @ ex
                rsb = pp.tile([128, 512], f32, tag="mm", bufs=4, name="psrs")
                for j in range(SJ):
                    sc = pp.tile([128, 512], f32, tag="mm", bufs=4, name="pssc")
                    nc.tensor.matmul(
                        sc[:], kts[g][:, bass.ts(j, 128)], qts[h][:],
                        start=True, stop=True,
                    )
                    ex = at.tile([128, 512], f32r, tag="expt", bufs=20, name="ex")
                    nc.scalar.activation(ex[:], sc[:], FP.Exp, scale=SCALE)
                    nc.tensor.matmul(
                        rsb[:], ones_sq[:], ex[:],
                        start=(j == 0), stop=(j == SJ - 1),
                    )
                    nc.tensor.matmul(
                        pv[:], vts[j][:, bass.ts(g, 128)], ex[:],
                        start=(j == 0), stop=(j == SJ - 1),
                    )
                recipb = at.tile([128, 512], f32, tag="recipb", bufs=2,
                                 name="rc")
                with nc.allow_low_precision(reason="1/rowsum feeds f32r mul"):
                    nc.vector.reciprocal(recipb[:], rsb[:])
                nc.vector.tensor_tensor(onorm[h][:, :], pv[:], recipb[:],
                                        op=ALU.mult)

            # ---------------- output projection ----------------------------
            with tc.tile_pool(name="oproj", bufs=1) as ot:
                for n in range(4):
                    psos = [pp.tile([128, 512], f32, tag="acc4", bufs=4,
                                    name=f"pso{i}") for i in range(4)]
                    for h in range(NH):
                        wo_t = ot.tile([128, 512], f32r, tag="wo", bufs=7,
                                       name="wot")
                        nc.sync.dma_start(
                            wo_t[:], woT[bass.ts(h, 128), bass.ts(n, 512)]
                        )
                        for sqt in range(4):
                            nc.tensor.matmul(
                                psos[sqt][:],
                                onorm[h][:, bass.ts(sqt, 128)], wo_t[:],
                                start=(h == 0), stop=(h == NH - 1),
                            )
                    for sqt in range(4):
                        o_s = ot.tile([128, 512], f32, tag="osb", bufs=2,
                                      name="osb")
                        nc.scalar.copy(o_s[:], psos[sqt][:])
                        nc.sync.dma_start(
                            outD[bass.ts(sqt, 128), bass.ts(n, 512)], o_s[:]
                        )


def build(reps=1):
    nc = bacc.Bacc("TRN2", target_bir_lowering=False, debug=False,
                   num_devices=NC)
    t = {
        "hT": nc.dram_tensor("hT", [H, S], f32r, kind="ExternalInput").ap(),
        "wqT": nc.dram_tensor("wqT", [H, H], f32r, kind="ExternalInput").ap(),
        "wkT": nc.dram_tensor("wkT", [H, NKV * HD], f32r,
                              kind="ExternalInput").ap(),
        "wvT": nc.dram_tensor("wvT", [H, NKV * HD], f32r,
                              kind="ExternalInput").ap(),
        "woT": nc.dram_tensor("woT", [H, H], f32r, kind="ExternalInput").ap(),
        "cc": nc.dram_tensor("cc", [128, S], f32, kind="ExternalInput").ap(),
        "ss": nc.dram_tensor("ss", [128, S], f32, kind="ExternalInput").ap(),
        "out": nc.dram_tensor("out", [SQ, H], f32, kind="ExternalOutput").ap(),
    }
    with tile.TileContext(nc) as tc:
        for _ in range(reps):
            _body(nc, tc, t)
    nc.compile()
    return nc


_ROPE_PERM = np.concatenate(
    [h * HD + np.r_[np.arange(0, HD, 2), np.arange(1, HD, 2)]
     for h in range(NH)]
)
_ROPE_PERM_KV = _ROPE_PERM[: NKV * HD]


def prep_inputs(hidden_states, freqs_cos, freqs_sin, Wq, Wk, Wv, Wo):
    """Host-side layout prep -> list of 8 per-core input maps."""
    wqT = np.ascontiguousarray(Wq.T[:, _ROPE_PERM])
    wkT = np.ascontiguousarray(Wk.T[:, _ROPE_PERM_KV])
    wvT = np.ascontiguousarray(Wv.T)
    woT = np.ascontiguousarray(Wo.T)
    cosT = freqs_cos.T  # [64, S]
    sinT = freqs_sin.T
    in_maps = []
    for c in range(NC):
        b, chunk = divmod(c, 4)
        sq0 = chunk * SQ
        perm = np.r_[sq0: sq0 + SQ, 0:sq0, sq0 + SQ: S]
        hTc = np.ascontiguousarray(hidden_states[b].T[:, perm])
        ccc = np.ascontiguousarray(np.concatenate([cosT, cosT], 0)[:, perm])
        ssc = np.ascontiguousarray(np.concatenate([sinT, sinT], 0)[:, perm])
        in_maps.append(
            {"hT": hTc, "wqT": wqT, "wkT": wkT, "wvT": wvT, "woT": woT,
             "cc": ccc, "ss": ssc}
        )
    return in_maps


_CACHE = {}


def _get_nc(reps=1):
    if reps not in _CACHE:
        _CACHE[reps] = build(reps)
    return _CACHE[reps]


def kernel(hidden_states, freqs_cos, freqs_sin, Wq, Wk, Wv, Wo):
    in_maps = prep_inputs(
        np.asarray(hidden_states, np.float32),
        np.asarray(freqs_cos, np.float32),
        np.asarray(freqs_sin, np.float32),
        np.asarray(Wq, np.float32),
        np.asarray(Wk, np.float32),
        np.asarray(Wv, np.float32),
        np.asarray(Wo, np.float32),
    )
    nc = _get_nc(int(os.environ.get("KERNEL_REPS", "1")))
    res = run_bass_kernel_spmd(nc, in_maps, core_ids=list(range(NC)))
    out = np.empty((B, S, H), np.float32)
    for c in range(NC):
        b, chunk = divmod(c, 4)
        out[b, chunk * SQ: (chunk + 1) * SQ, :] = res.results[c]["out"]
    return out



# revision 2
# speedup vs baseline: 1.2072x; 1.2072x over previous
"""Trainium2 Bass kernel for nn_CustomAttentionLayer (GQA attention + RoPE + o_proj).

Sharding: head-parallel, 8-way over (batch, kv-head-group): core c handles
batch c//4 and kv head g=c%4, i.e. query heads 4g..4g+3. Each core computes
q/k/v projections for exactly its heads (zero redundant compute), attention
over the full 2048x2048 score matrix for its 4 query heads, and a PARTIAL
output projection (contribution of its heads to the full [S,H] output).
The host sums the 4 partials per batch — no on-device collectives.

All matmul operands are bfloat16 (fp32 accumulate in PSUM): same TensorE
throughput as float32r in this regime but no small-free-dim penalty, half
the DMA traffic, and 2x DVE throughput for the softmax-denominator
accumulation. Measured end-to-end rel err ~8e-3 (budget 2e-2).

Softmax skips max-subtraction (scores are O(+-5)); the row sums come from a
running DVE accumulation of the bf16 exp tiles (pair adds at 2x DVE rate)
plus ONE ones-matmul per (chunk, head) instead of one per key-tile, cutting
the PE rowsum cost by 16x.
"""

import os
import numpy as np
import ml_dtypes

import concourse.bass as bass
import concourse.mybir as mybir
import concourse.tile as tile
from concourse import bacc
from concourse.bass_utils import run_bass_kernel_spmd

B, S, H = 2, 2048, 2048
NH, NKV, HD = 16, 4, 128
HQ = NH // NKV                # 4 query heads per core
NC = 8                        # cores
KT = H // 128                 # 16 contraction tiles over H
CH = 512                      # query-chunk width (PSUM bank limit)
NCH = S // CH                 # 4 chunks
SJ = S // 128                 # 16 key-position tiles
SCALE = 1.0 / float(np.sqrt(HD))

f32 = mybir.dt.float32
bf16 = mybir.dt.bfloat16
FP = mybir.ActivationFunctionType
ALU = mybir.AluOpType


def _body(nc, tc, t):
    hT, wq, wk, wv, wo, ccD, ssD, outD = (
        t["hT"], t["wq"], t["wk"], t["wv"], t["wo"], t["cc"], t["ss"], t["out"]
    )
    with tc.tile_pool(name="main", bufs=1) as main, tc.tile_pool(
        name="psum", bufs=1, space="PSUM"
    ) as pp:
        # ---------------- persistent SBUF tensors ---------------------------
        cc = main.tile([128, S], bf16, tag="cc", bufs=1)
        ss = main.tile([128, S], bf16, tag="ss", bufs=1)
        wk_s = main.tile([128, KT * 128], bf16, tag="wk", bufs=1)
        wv_s = main.tile([128, KT * 128], bf16, tag="wv", bufs=1)
        wq_s = main.tile([128, KT * HQ * 128], bf16, tag="wq", bufs=1)
        wo_s = main.tile([128, HQ * H], bf16, tag="wo", bufs=1)
        qts = [main.tile([128, S], bf16, tag="qt", bufs=HQ, name=f"qt{i}")
               for i in range(HQ)]
        kt = main.tile([128, S], bf16, tag="kt", bufs=1)
        vts = [main.tile([128, HD], bf16, tag="v", bufs=SJ, name=f"v{i}")
               for i in range(SJ)]
        hq = [[main.tile([128, CH], bf16, tag="hq", bufs=KT * NCH,
                         name=f"hq{c}_{k}") for k in range(KT)]
              for c in range(NCH)]

        # DMA order matters at startup: k-proj (wk + hq chunk 0 + rope
        # tables) first, then the rest.
        nc.sync.dma_start(wk_s[:], wk)
        for k in range(KT):
            nc.sync.dma_start(hq[0][k][:], hT[bass.ts(k, 128), bass.ts(0, CH)])
        nc.sync.dma_start(cc[:], ccD)
        nc.sync.dma_start(ss[:], ssD)
        for c in range(1, NCH):
            for k in range(KT):
                nc.sync.dma_start(hq[c][k][:],
                                  hT[bass.ts(k, 128), bass.ts(c, CH)])
        nc.sync.dma_start(wv_s[:], wv)
        for part in range(4):
            w = KT * HQ * 128 // 4
            nc.sync.dma_start(wq_s[:, part * w:(part + 1) * w],
                              wq[:, part * w:(part + 1) * w])
        nc.sync.dma_start(wo_s[:], wo)

        def rope(dst, ps, cols, pool):
            # x = [xr; xi] on partition halves, cc = [c; c], ss = [s; s]:
            #   dst[0:64]   = xr*c - xi*s
            #   dst[64:128] = xr*s + xi*c
            w = dst.shape[-1]
            t1 = pool.tile([64, w], f32, tag="ropeA", bufs=2, name="t1")
            t2 = pool.tile([64, w], f32, tag="ropeB", bufs=2, name="t2")
            nc.vector.tensor_tensor(t1[:], ps[64:128, :], ss[64:128, cols],
                                    op=ALU.mult)
            nc.vector.tensor_tensor(t2[:], ps[0:64, :], cc[0:64, cols],
                                    op=ALU.mult)
            nc.vector.tensor_sub(dst[0:64, :], t2[:], t1[:])
            nc.vector.tensor_tensor(t1[:], ps[0:64, :], ss[0:64, cols],
                                    op=ALU.mult)
            nc.vector.tensor_tensor(t2[:], ps[64:128, :], cc[64:128, cols],
                                    op=ALU.mult)
            nc.vector.tensor_add(dst[64:128, :], t2[:], t1[:])

        with tc.tile_pool(name="work", bufs=1) as wp:
            # ---------------- k projection (all chunks) ---------------------
            for c in range(NCH):
                cols = bass.ts(c, CH)
                ps = pp.tile([128, CH], f32, tag="mm", bufs=4, name="psk")
                for k in range(KT):
                    nc.tensor.matmul(
                        ps[:], wk_s[:, bass.ts(k, 128)], hq[c][k][:],
                        start=(k == 0), stop=(k == KT - 1),
                    )
                rope(kt[:, cols], ps, cols, wp)

            # ---------------- v projection (all chunks), [kp, d] layout -----
            for c in range(NCH):
                pv4 = pp.tile([128, 4 * HD], f32, tag="vp", bufs=2, name="psv")
                for sub in range(4):
                    for k in range(KT):
                        nc.tensor.matmul(
                            pv4[:, bass.ts(sub, HD)],
                            hq[c][k][:, bass.ts(sub, 128)],
                            wv_s[:, bass.ts(k, 128)],
                            start=(k == 0), stop=(k == KT - 1),
                        )
                for sub in range(4):
                    nc.scalar.copy(vts[c * 4 + sub][:], pv4[:, bass.ts(sub, HD)])

            # ---------------- q projection (all chunks, 4 heads) ------------
            for c in range(NCH):
                cols = bass.ts(c, CH)
                for m in range(HQ):
                    ps = pp.tile([128, CH], f32, tag="mm", bufs=4, name="psq")
                    for k in range(KT):
                        nc.tensor.matmul(
                            ps[:],
                            wq_s[:, k * HQ * 128 + m * 128:
                                 k * HQ * 128 + (m + 1) * 128],
                            hq[c][k][:],
                            start=(k == 0), stop=(k == KT - 1),
                        )
                    rope(qts[m][:, cols], ps, cols, wp)

        # ---------------- attention + output projection ---------------------
        with tc.tile_pool(name="attn", bufs=1) as at:
            ones_b = at.tile([128, 128], bf16, tag="ones", bufs=1)
            nc.vector.memset(ones_b[:], 1.0)

            onorm = [[at.tile([128, CH], bf16, tag="onorm", bufs=2 * HQ,
                              name=f"on{c}_{m}") for m in range(HQ)]
                     for c in range(NCH)]

            def attn_chunk(c, m):
                cols = bass.ts(c, CH)
                pv = pp.tile([128, CH], f32, tag="acc", bufs=2, name="pspv")
                acc = None
                for j in range(SJ):
                    sc = pp.tile([128, CH], f32, tag="mm", bufs=4, name="pssc")
                    nc.tensor.matmul(
                        sc[:], kt[:, bass.ts(j, 128)], qts[m][:, cols],
                        start=True, stop=True,
                    )
                    ex = at.tile([128, CH], bf16, tag="expt", bufs=8, name="ex")
                    nc.scalar.activation(ex[:], sc[:], FP.Exp, scale=SCALE)
                    nc.tensor.matmul(
                        pv[:], vts[j][:], ex[:],
                        start=(j == 0), stop=(j == SJ - 1),
                    )
                    if j == 0:
                        acc = ex
                    else:
                        nacc = at.tile([128, CH], bf16, tag="exacc", bufs=2,
                                       name="exacc")
                        nc.vector.tensor_add(nacc[:], acc[:], ex[:])
                        acc = nacc
                # rowsum broadcast to 128 partitions via ones-matmul
                rsb = pp.tile([128, CH], f32, tag="mm", bufs=4, name="psrs")
                nc.tensor.matmul(rsb[:], ones_b[:], acc[:], start=True,
                                 stop=True)
                recipb = at.tile([128, CH], f32, tag="recipb", bufs=2,
                                 name="rc")
                with nc.allow_low_precision(reason="1/rowsum feeds bf16 mul"):
                    nc.vector.reciprocal(recipb[:], rsb[:])
                nc.vector.tensor_tensor(onorm[c][m][:], pv[:], recipb[:],
                                        op=ALU.mult)

            def oproj_chunk(c):
                for sub in range(4):
                    for n in range(4):
                        po = pp.tile([128, CH], f32, tag="mm", bufs=4,
                                     name="pso")
                        for m in range(HQ):
                            nc.tensor.matmul(
                                po[:],
                                onorm[c][m][:, bass.ts(sub, 128)],
                                wo_s[:, m * H + n * CH: m * H + (n + 1) * CH],
                                start=(m == 0), stop=(m == HQ - 1),
                            )
                        o_s = at.tile([128, CH], f32, tag="osb", bufs=4,
                                      name="osb")
                        nc.scalar.copy(o_s[:], po[:])
                        nc.sync.dma_start(
                            outD[bass.ts(c * 4 + sub, 128), bass.ts(n, CH)],
                            o_s[:],
                        )

            for c in range(NCH):
                for m in range(HQ):
                    attn_chunk(c, m)
                if c > 0:
                    oproj_chunk(c - 1)
            oproj_chunk(NCH - 1)


def build(reps=1):
    nc = bacc.Bacc("TRN2", target_bir_lowering=False, debug=False,
                   num_devices=NC)
    t = {
        "hT": nc.dram_tensor("hT", [H, S], bf16, kind="ExternalInput").ap(),
        "wq": nc.dram_tensor("wq", [128, KT * HQ * 128], bf16,
                             kind="ExternalInput").ap(),
        "wk": nc.dram_tensor("wk", [128, KT * 128], bf16,
                             kind="ExternalInput").ap(),
        "wv": nc.dram_tensor("wv", [128, KT * 128], bf16,
                             kind="ExternalInput").ap(),
        "wo": nc.dram_tensor("wo", [128, HQ * H], bf16,
                             kind="ExternalInput").ap(),
        "cc": nc.dram_tensor("cc", [128, S], bf16, kind="ExternalInput").ap(),
        "ss": nc.dram_tensor("ss", [128, S], bf16, kind="ExternalInput").ap(),
        "out": nc.dram_tensor("out", [S, H], f32, kind="ExternalOutput").ap(),
    }
    with tile.TileContext(nc) as tc:
        for _ in range(reps):
            _body(nc, tc, t)
    nc.compile()
    return nc


# per-head rope permutation: [even dims, odd dims]
_RP = np.r_[np.arange(0, HD, 2), np.arange(1, HD, 2)]


def _bf(x):
    return np.ascontiguousarray(x).astype(ml_dtypes.bfloat16)


def prep_inputs(hidden_states, freqs_cos, freqs_sin, Wq, Wk, Wv, Wo):
    """Host-side layout prep -> list of 8 per-core input maps."""
    cosT = np.concatenate([freqs_cos.T, freqs_cos.T], 0)  # [128, S]
    sinT = np.concatenate([freqs_sin.T, freqs_sin.T], 0)
    cc = _bf(cosT)
    ss = _bf(sinT)
    wqT = Wq.T    # [H, H]
    wkT = Wk.T    # [H, 512]
    wvT = Wv.T
    woT = Wo.T    # [H, H]
    hT = [_bf(hidden_states[b].T) for b in range(B)]
    in_maps = []
    for c in range(NC):
        b, g = divmod(c, HQ)
        # wq: [p, k, m, j] -> [128, KT*HQ*128]; head cols rope-permuted
        cols = np.concatenate(
            [(4 * g + m) * HD + _RP for m in range(HQ)])      # [512]
        wq_p = wqT[:, cols].reshape(KT, 128, HQ * 128).transpose(1, 0, 2)
        wk_p = wkT[:, g * HD + _RP].reshape(KT, 128, 128).transpose(1, 0, 2)
        wv_p = wvT[:, g * HD: (g + 1) * HD].reshape(
            KT, 128, 128).transpose(1, 0, 2)
        wo_p = woT[4 * g * HD: (4 * g + 4) * HD, :].reshape(
            HQ, 128, H).transpose(1, 0, 2)
        in_maps.append({
            "hT": hT[b],
            "wq": _bf(wq_p.reshape(128, -1)),
            "wk": _bf(wk_p.reshape(128, -1)),
            "wv": _bf(wv_p.reshape(128, -1)),
            "wo": _bf(wo_p.reshape(128, -1)),
            "cc": cc, "ss": ss,
        })
    return in_maps


_CACHE = {}


def _get_nc(reps=1):
    if reps not in _CACHE:
        _CACHE[reps] = build(reps)
    return _CACHE[reps]


def kernel(hidden_states, freqs_cos, freqs_sin, Wq, Wk, Wv, Wo):
    in_maps = prep_inputs(
        np.asarray(hidden_states, np.float32),
        np.asarray(freqs_cos, np.float32),
        np.asarray(freqs_sin, np.float32),
        np.asarray(Wq, np.float32),
        np.asarray(Wk, np.float32),
        np.asarray(Wv, np.float32),
        np.asarray(Wo, np.float32),
    )
    nc = _get_nc(int(os.environ.get("KERNEL_REPS", "1")))
    res = run_bass_kernel_spmd(nc, in_maps, core_ids=list(range(NC)))
    out = np.zeros((B, S, H), np.float32)
    for c in range(NC):
        b = c // HQ
        out[b] += np.asarray(res.results[c]["out"], np.float32)
    return out


# revision 4
# speedup vs baseline: 1.3549x; 1.1223x over previous
"""Trainium2 Bass kernel for nn_CustomAttentionLayer (GQA attention + RoPE + o_proj).

Sharding: head-parallel, 8-way over (batch, kv-head-group): core c handles
batch c//4 and kv head g=c%4, i.e. query heads 4g..4g+3. Each core computes
q/k/v projections for exactly its heads (zero redundant compute), attention
over the full 2048x2048 score matrix for its 4 query heads, and a PARTIAL
output projection (contribution of its heads to the full [S,H] output).
The host sums the 4 partials per batch — no on-device collectives.

All matmul operands are bfloat16 (fp32 accumulate in PSUM): same TensorE
throughput as float32r in this regime but no small-free-dim penalty, half
the DMA traffic, and 2x DVE throughput for the softmax-denominator
accumulation. Measured end-to-end rel err ~7e-3 (budget 2e-2).

Schedule: the attention j-loop is software-pipelined (pv lags sc by one
iteration to hide the exp latency) and the q-projection of the NEXT chunk
plus the output projection of the PREVIOUS chunk are interleaved one matmul
per j-iteration as PE filler, so the TensorEngine never starves while the
Activation engine streams the 256 exp tiles. Row sums come from a running
DVE bf16 accumulation (2x rate) plus ONE ones-matmul per (chunk, head).
o_proj partials are DMA'd to DRAM directly from PSUM.
"""

import os
import numpy as np
import ml_dtypes

import concourse.bass as bass
import concourse.mybir as mybir
import concourse.tile as tile
from concourse import bacc
from concourse.bass_utils import run_bass_kernel_spmd

B, S, H = 2, 2048, 2048
NH, NKV, HD = 16, 4, 128
HQ = NH // NKV                # 4 query heads per core
NC = 8                        # cores
KT = H // 128                 # 16 contraction tiles over H
CH = 512                      # query-chunk width (PSUM bank limit)
NCH = S // CH                 # 4 chunks
SJ = S // 128                 # 16 key-position tiles
SCALE = 1.0 / float(np.sqrt(HD))

f32 = mybir.dt.float32
bf16 = mybir.dt.bfloat16
FP = mybir.ActivationFunctionType
ALU = mybir.AluOpType


def _body(nc, tc, t):
    hT, wq, wk, wv, wo, ccD, ssD, outD = (
        t["hT"], t["wq"], t["wk"], t["wv"], t["wo"], t["cc"], t["ss"], t["out"]
    )
    with tc.tile_pool(name="main", bufs=1) as main, tc.tile_pool(
        name="psum", bufs=1, space="PSUM"
    ) as pp:
        # ---------------- persistent SBUF tensors ---------------------------
        cc = main.tile([128, S], bf16, tag="cc", bufs=1)
        ss = main.tile([128, S], bf16, tag="ss", bufs=1)
        wk_s = main.tile([128, KT * 128], bf16, tag="wk", bufs=1)
        wv_s = main.tile([128, KT * 128], bf16, tag="wv", bufs=1)
        wq_s = main.tile([128, KT * HQ * 128], bf16, tag="wq", bufs=1)
        wo_s = main.tile([128, HQ * H], bf16, tag="wo", bufs=1)
        qts = [[main.tile([128, CH], bf16, tag="qt", bufs=HQ * NCH,
                          name=f"qt{m}_{c}") for c in range(NCH)]
               for m in range(HQ)]
        kt = main.tile([128, S], bf16, tag="kt", bufs=1)
        vts = [main.tile([128, HD], bf16, tag="v", bufs=SJ, name=f"v{i}")
               for i in range(SJ)]
        hq = [[main.tile([128, CH], bf16, tag="hq", bufs=KT * NCH,
                         name=f"hq{c}_{k}") for k in range(KT)]
              for c in range(NCH)]

        # DMA order matters at startup: k-proj (wk + hq chunk 0 + rope
        # tables) first, then the rest.
        nc.sync.dma_start(wk_s[:], wk)
        for k in range(KT):
            nc.sync.dma_start(hq[0][k][:], hT[bass.ts(k, 128), bass.ts(0, CH)])
        nc.sync.dma_start(cc[:], ccD)
        nc.sync.dma_start(ss[:], ssD)
        for c in range(1, NCH):
            for k in range(KT):
                nc.sync.dma_start(hq[c][k][:],
                                  hT[bass.ts(k, 128), bass.ts(c, CH)])
        nc.sync.dma_start(wv_s[:], wv)
        for part in range(4):
            w = KT * HQ * 128 // 4
            nc.sync.dma_start(wq_s[:, part * w:(part + 1) * w],
                              wq[:, part * w:(part + 1) * w])
        nc.sync.dma_start(wo_s[:], wo)

        def rope(dst, ps, cols, pool):
            # x = [xr; xi] on partition halves, cc = [c; c], ss = [s; s]:
            #   dst[0:64]   = xr*c - xi*s
            #   dst[64:128] = xr*s + xi*c
            w = dst.shape[-1]
            t1 = pool.tile([64, w], f32, tag="ropeA", bufs=2, name="t1")
            t2 = pool.tile([64, w], f32, tag="ropeB", bufs=2, name="t2")
            nc.vector.tensor_tensor(t1[:], ps[64:128, :], ss[64:128, cols],
                                    op=ALU.mult)
            nc.vector.tensor_tensor(t2[:], ps[0:64, :], cc[0:64, cols],
                                    op=ALU.mult)
            nc.vector.tensor_sub(dst[0:64, :], t2[:], t1[:])
            nc.vector.tensor_tensor(t1[:], ps[0:64, :], ss[0:64, cols],
                                    op=ALU.mult)
            nc.vector.tensor_tensor(t2[:], ps[64:128, :], cc[64:128, cols],
                                    op=ALU.mult)
            nc.vector.tensor_add(dst[64:128, :], t2[:], t1[:])

        def qproj_mm(c, m, k, wp):
            """One contraction-tile matmul of q-projection (c, m)."""
            if k == 0:
                qproj_mm.ps = pp.tile([128, CH], f32, tag="psq", bufs=2,
                                      name="psq")
            nc.tensor.matmul(
                qproj_mm.ps[:],
                wq_s[:, k * HQ * 128 + m * 128: k * HQ * 128 + (m + 1) * 128],
                hq[c][k][:],
                start=(k == 0), stop=(k == KT - 1),
            )
            if k == KT - 1:
                rope(qts[m][c][:], qproj_mm.ps, bass.ts(c, CH), wp)

        with tc.tile_pool(name="work", bufs=1) as wp:
            # ---------------- k projection (all chunks) ---------------------
            for c in range(NCH):
                cols = bass.ts(c, CH)
                ps = pp.tile([128, CH], f32, tag="mm", bufs=2, name="psk")
                for k in range(KT):
                    nc.tensor.matmul(
                        ps[:], wk_s[:, bass.ts(k, 128)], hq[c][k][:],
                        start=(k == 0), stop=(k == KT - 1),
                    )
                rope(kt[:, cols], ps, cols, wp)

            # ---------------- v projection (all chunks), [kp, d] layout -----
            for c in range(NCH):
                pv4 = pp.tile([128, 4 * HD], f32, tag="po", bufs=2, name="psv")
                for sub in range(4):
                    for k in range(KT):
                        nc.tensor.matmul(
                            pv4[:, bass.ts(sub, HD)],
                            hq[c][k][:, bass.ts(sub, 128)],
                            wv_s[:, bass.ts(k, 128)],
                            start=(k == 0), stop=(k == KT - 1),
                        )
                for sub in range(4):
                    nc.scalar.copy(vts[c * 4 + sub][:], pv4[:, bass.ts(sub, HD)])

            # ---------------- q projection chunk 0 --------------------------
            for m in range(HQ):
                for k in range(KT):
                    qproj_mm(0, m, k, wp)

            # ---------------- attention + interleaved q/o-proj --------------
            ones_b = wp.tile([128, 128], bf16, tag="ones", bufs=1)
            nc.vector.memset(ones_b[:], 1.0)

            onorm = [[wp.tile([128, CH], bf16, tag="onorm", bufs=2 * HQ,
                              name=f"on{c}_{m}") for m in range(HQ)]
                     for c in range(NCH)]

            def oproj_mm(c, sub, n, m):
                """One head-contribution matmul of o-proj group (c, sub, n)."""
                if m == 0:
                    oproj_mm.po = pp.tile([128, CH], f32, tag="po", bufs=2,
                                          name="pso")
                nc.tensor.matmul(
                    oproj_mm.po[:],
                    onorm[c][m][:, bass.ts(sub, 128)],
                    wo_s[:, m * H + n * CH: m * H + (n + 1) * CH],
                    start=(m == 0), stop=(m == HQ - 1),
                )
                if m == HQ - 1:
                    o_s = wp.tile([128, CH], f32, tag="osb", bufs=4,
                                  name="osb")
                    nc.scalar.copy(o_s[:], oproj_mm.po[:])
                    nc.sync.dma_start(
                        outD[bass.ts(c * 4 + sub, 128), bass.ts(n, CH)],
                        o_s[:],
                    )

            def attn_group(c, mi):
                pv = pp.tile([128, CH], f32, tag="acc", bufs=2, name="pspv")
                scs = [None] * SJ
                exs = [None] * SJ
                acc = None
                for j in range(SJ):
                    sc = pp.tile([128, CH], f32, tag="mm", bufs=2, name="pssc")
                    scs[j] = sc
                    nc.tensor.matmul(
                        sc[:], kt[:, bass.ts(j, 128)], qts[mi][c][:],
                        start=True, stop=True,
                    )
                    ex = wp.tile([128, CH], bf16, tag="expt", bufs=8,
                                 name="ex")
                    exs[j] = ex
                    nc.scalar.activation(ex[:], sc[:], FP.Exp, scale=SCALE)
                    if j == 0:
                        acc = ex
                    else:
                        nacc = wp.tile([128, CH], bf16, tag="exacc", bufs=2,
                                       name="exacc")
                        nc.vector.tensor_add(nacc[:], acc[:], ex[:])
                        acc = nacc
                    # filler: q-projection of next chunk, same head slot
                    if c < NCH - 1:
                        qproj_mm(c + 1, mi, j, wp)
                    # pv lags sc by one iteration (hides exp latency)
                    if j >= 1:
                        nc.tensor.matmul(
                            pv[:], vts[j - 1][:], exs[j - 1][:],
                            start=(j == 1), stop=False,
                        )
                    # filler: o-projection of previous chunk
                    if c > 0:
                        oproj_mm(c - 1, mi, j // 4, j % 4)
                nc.tensor.matmul(pv[:], vts[SJ - 1][:], exs[SJ - 1][:],
                                 start=False, stop=True)
                rsb = pp.tile([128, CH], f32, tag="mm", bufs=2, name="psrs")
                nc.tensor.matmul(rsb[:], ones_b[:], acc[:], start=True,
                                 stop=True)
                recipb = wp.tile([128, CH], f32, tag="recipb", bufs=2,
                                 name="rc")
                with nc.allow_low_precision(reason="1/rowsum feeds bf16 mul"):
                    nc.vector.reciprocal(recipb[:], rsb[:])
                nc.vector.tensor_tensor(onorm[c][mi][:], pv[:], recipb[:],
                                        op=ALU.mult)

            for c in range(NCH):
                for mi in range(HQ):
                    attn_group(c, mi)
            # tail: o-proj of the last chunk
            for sub in range(4):
                for n in range(4):
                    for m in range(HQ):
                        oproj_mm(NCH - 1, sub, n, m)


def build(reps=1):
    nc = bacc.Bacc("TRN2", target_bir_lowering=False, debug=False,
                   num_devices=NC)
    t = {
        "hT": nc.dram_tensor("hT", [H, S], bf16, kind="ExternalInput").ap(),
        "wq": nc.dram_tensor("wq", [128, KT * HQ * 128], bf16,
                             kind="ExternalInput").ap(),
        "wk": nc.dram_tensor("wk", [128, KT * 128], bf16,
                             kind="ExternalInput").ap(),
        "wv": nc.dram_tensor("wv", [128, KT * 128], bf16,
                             kind="ExternalInput").ap(),
        "wo": nc.dram_tensor("wo", [128, HQ * H], bf16,
                             kind="ExternalInput").ap(),
        "cc": nc.dram_tensor("cc", [128, S], bf16, kind="ExternalInput").ap(),
        "ss": nc.dram_tensor("ss", [128, S], bf16, kind="ExternalInput").ap(),
        "out": nc.dram_tensor("out", [S, H], f32, kind="ExternalOutput").ap(),
    }
    with tile.TileContext(nc) as tc:
        for _ in range(reps):
            _body(nc, tc, t)
    nc.compile()
    return nc


# per-head rope permutation: [even dims, odd dims]
_RP = np.r_[np.arange(0, HD, 2), np.arange(1, HD, 2)]


def _bf(x):
    return np.ascontiguousarray(x).astype(ml_dtypes.bfloat16)


def prep_inputs(hidden_states, freqs_cos, freqs_sin, Wq, Wk, Wv, Wo):
    """Host-side layout prep -> list of 8 per-core input maps."""
    cosT = np.concatenate([freqs_cos.T, freqs_cos.T], 0)  # [128, S]
    sinT = np.concatenate([freqs_sin.T, freqs_sin.T], 0)
    cc = _bf(cosT)
    ss = _bf(sinT)
    wqT = Wq.T    # [H, H]
    wkT = Wk.T    # [H, 512]
    wvT = Wv.T
    woT = Wo.T    # [H, H]
    hT = [_bf(hidden_states[b].T) for b in range(B)]
    in_maps = []
    for c in range(NC):
        b, g = divmod(c, HQ)
        # wq: [p, k, m, j] -> [128, KT*HQ*128]; head cols rope-permuted
        cols = np.concatenate(
            [(4 * g + m) * HD + _RP for m in range(HQ)])      # [512]
        wq_p = wqT[:, cols].reshape(KT, 128, HQ * 128).transpose(1, 0, 2)
        wk_p = wkT[:, g * HD + _RP].reshape(KT, 128, 128).transpose(1, 0, 2)
        wv_p = wvT[:, g * HD: (g + 1) * HD].reshape(
            KT, 128, 128).transpose(1, 0, 2)
        wo_p = woT[4 * g * HD: (4 * g + 4) * HD, :].reshape(
            HQ, 128, H).transpose(1, 0, 2)
        in_maps.append({
            "hT": hT[b],
            "wq": _bf(wq_p.reshape(128, -1)),
            "wk": _bf(wk_p.reshape(128, -1)),
            "wv": _bf(wv_p.reshape(128, -1)),
            "wo": _bf(wo_p.reshape(128, -1)),
            "cc": cc, "ss": ss,
        })
    return in_maps


_CACHE = {}


def _get_nc(reps=1):
    if reps not in _CACHE:
        _CACHE[reps] = build(reps)
    return _CACHE[reps]


def kernel(hidden_states, freqs_cos, freqs_sin, Wq, Wk, Wv, Wo):
    in_maps = prep_inputs(
        np.asarray(hidden_states, np.float32),
        np.asarray(freqs_cos, np.float32),
        np.asarray(freqs_sin, np.float32),
        np.asarray(Wq, np.float32),
        np.asarray(Wk, np.float32),
        np.asarray(Wv, np.float32),
        np.asarray(Wo, np.float32),
    )
    nc = _get_nc(int(os.environ.get("KERNEL_REPS", "1")))
    res = run_bass_kernel_spmd(nc, in_maps, core_ids=list(range(NC)))
    out = np.zeros((B, S, H), np.float32)
    for c in range(NC):
        b = c // HQ
        out[b] += np.asarray(res.results[c]["out"], np.float32)
    return out


# revision 10
# speedup vs baseline: 1.4058x; 1.0376x over previous
"""Trainium2 Bass kernel for nn_CustomAttentionLayer (GQA attention + RoPE + o_proj).

Sharding: head-parallel, 8-way over (batch, kv-head-group): core c handles
batch c//4 and kv head g=c%4, i.e. query heads 4g..4g+3. Each core computes
q/k/v projections for exactly its heads (zero redundant compute), attention
over the full 2048x2048 score matrix for its 4 query heads, and a PARTIAL
output projection (contribution of its heads to the full [S,H] output).
The host sums the 4 partials per batch — no on-device collectives.

All matmul operands are bfloat16 (fp32 accumulate in PSUM): same TensorE
throughput as float32r in this regime but no small-free-dim penalty, half
the DMA traffic, and 2x DVE throughput for the softmax-denominator
accumulation. Measured end-to-end rel err ~7e-3 (budget 2e-2).

Schedule: the attention j-loop is software-pipelined (pv lags sc by one
iteration to hide the exp latency) and the q-projection of the NEXT chunk
plus the output projection of the PREVIOUS chunk are interleaved one matmul
per j-iteration as PE filler, so the TensorEngine never starves while the
Activation engine streams the 256 exp tiles. Row sums come from a running
DVE bf16 accumulation (2x rate) plus ONE ones-matmul per (chunk, head).
o_proj partials are DMA'd to DRAM directly from PSUM.
"""

import os
import numpy as np
import ml_dtypes

import concourse.bass as bass
import concourse.mybir as mybir
import concourse.tile as tile
from concourse import bacc
from concourse.bass_utils import run_bass_kernel_spmd

B, S, H = 2, 2048, 2048
NH, NKV, HD = 16, 4, 128
HQ = NH // NKV                # 4 query heads per core
NC = 8                        # cores
KT = H // 128                 # 16 contraction tiles over H
CH = 512                      # query-chunk width (PSUM bank limit)
NCH = S // CH                 # 4 chunks
SJ = S // 128                 # 16 key-position tiles
SCALE = 1.0 / float(np.sqrt(HD))

f32 = mybir.dt.float32
bf16 = mybir.dt.bfloat16
FP = mybir.ActivationFunctionType
ALU = mybir.AluOpType


def _body(nc, tc, t):
    hT, wq, wk, wv, wo, ccD, ssD, outD = (
        t["hT"], t["wq"], t["wk"], t["wv"], t["wo"], t["cc"], t["ss"], t["out"]
    )
    with tc.tile_pool(name="main", bufs=1) as main, tc.tile_pool(
        name="psum", bufs=1, space="PSUM"
    ) as pp:
        # ---------------- persistent SBUF tensors ---------------------------
        cc = main.tile([128, S], bf16, tag="cc", bufs=1)
        ss = main.tile([128, S], bf16, tag="ss", bufs=1)
        wk_s = main.tile([128, KT * 128], bf16, tag="wk", bufs=1)
        wv_s = main.tile([128, KT * 128], bf16, tag="wv", bufs=1)
        wq_s = main.tile([128, KT * HQ * 128], bf16, tag="wq", bufs=1)
        wo_s = main.tile([128, HQ * H], bf16, tag="wo", bufs=1)
        qts = [[main.tile([128, CH], bf16, tag="qt", bufs=HQ * NCH,
                          name=f"qt{m}_{c}") for c in range(NCH)]
               for m in range(HQ)]
        kt = main.tile([128, S], bf16, tag="kt", bufs=1)
        vts = [main.tile([128, HD], bf16, tag="v", bufs=SJ, name=f"v{i}")
               for i in range(SJ)]
        hq = [[main.tile([128, CH], bf16, tag="hq", bufs=KT * NCH,
                         name=f"hq{c}_{k}") for k in range(KT)]
              for c in range(NCH)]

        # DMA issue order ~ consumption order so the startup is never
        # DMA-paced: wk, h chunk 0, wv, rope tables, h chunks 1-3, wq
        # (split by contraction quarters, matching q-proj's k-loop), wo.
        def load_hq(c):
            for k in range(KT):
                nc.sync.dma_start(hq[c][k][:],
                                  hT[bass.ts(k, 128), bass.ts(c, CH)])

        nc.sync.dma_start(wk_s[:, :1024], wk[:, :1024])
        nc.sync.dma_start(wk_s[:, 1024:], wk[:, 1024:])
        load_hq(0)
        nc.sync.dma_start(wv_s[:], wv)
        nc.sync.dma_start(cc[:], ccD)
        nc.sync.dma_start(ss[:], ssD)
        load_hq(1)
        load_hq(2)
        load_hq(3)
        for part in range(4):
            w = KT * HQ * 128 // 4
            nc.sync.dma_start(wq_s[:, part * w:(part + 1) * w],
                              wq[:, part * w:(part + 1) * w])
        nc.sync.dma_start(wo_s[:], wo)

        def rope(dst, ps, cols, pool):
            # x = [xr; xi] on partition halves, cc = [c; c], ss = [s; s]:
            #   dst[0:64]   = xr*c - xi*s
            #   dst[64:128] = xr*s + xi*c
            w = dst.shape[-1]
            t1 = pool.tile([64, w], f32, tag="ropeA", bufs=2, name="t1")
            t2 = pool.tile([64, w], f32, tag="ropeB", bufs=2, name="t2")
            nc.vector.tensor_tensor(t1[:], ps[64:128, :], ss[64:128, cols],
                                    op=ALU.mult)
            nc.vector.tensor_tensor(t2[:], ps[0:64, :], cc[0:64, cols],
                                    op=ALU.mult)
            nc.vector.tensor_sub(dst[0:64, :], t2[:], t1[:])
            nc.vector.tensor_tensor(t1[:], ps[0:64, :], ss[0:64, cols],
                                    op=ALU.mult)
            nc.vector.tensor_tensor(t2[:], ps[64:128, :], cc[64:128, cols],
                                    op=ALU.mult)
            nc.vector.tensor_add(dst[64:128, :], t2[:], t1[:])

        def qproj_mm(c, m, k, wp):
            """One contraction-tile matmul of q-projection (c, m)."""
            if k == 0:
                qproj_mm.ps = pp.tile([128, CH], f32, tag="psq", bufs=2,
                                      name="psq")
            nc.tensor.matmul(
                qproj_mm.ps[:],
                wq_s[:, k * HQ * 128 + m * 128: k * HQ * 128 + (m + 1) * 128],
                hq[c][k][:],
                start=(k == 0), stop=(k == KT - 1),
            )
            if k == KT - 1:
                rope(qts[m][c][:], qproj_mm.ps, bass.ts(c, CH), wp)

        with tc.tile_pool(name="work", bufs=1) as wp:
            # ------- k+v projections, interleaved per chunk (DMA-paced) -----
            for c in range(NCH):
                cols = bass.ts(c, CH)
                ps = pp.tile([128, CH], f32, tag="mm", bufs=2, name="psk")
                for k in range(KT):
                    nc.tensor.matmul(
                        ps[:], wk_s[:, bass.ts(k, 128)], hq[c][k][:],
                        start=(k == 0), stop=(k == KT - 1),
                    )
                rope(kt[:, cols], ps, cols, wp)
                pv4 = pp.tile([128, 4 * HD], f32, tag="po", bufs=2, name="psv")
                for sub in range(4):
                    for k in range(KT):
                        nc.tensor.matmul(
                            pv4[:, bass.ts(sub, HD)],
                            hq[c][k][:, bass.ts(sub, 128)],
                            wv_s[:, bass.ts(k, 128)],
                            start=(k == 0), stop=(k == KT - 1),
                        )
                for sub in range(4):
                    nc.scalar.copy(vts[c * 4 + sub][:], pv4[:, bass.ts(sub, HD)])

            # ---------------- q projection chunk 0 --------------------------
            for m in range(HQ):
                for k in range(KT):
                    qproj_mm(0, m, k, wp)

            # ---------------- attention + interleaved q/o-proj --------------
            ones_b = wp.tile([128, 128], bf16, tag="ones", bufs=1)
            nc.vector.memset(ones_b[:], 1.0)

            onorm = [[wp.tile([128, CH], bf16, tag="onorm", bufs=2 * HQ,
                              name=f"on{c}_{m}") for m in range(HQ)]
                     for c in range(NCH)]

            def oproj_mm(c, sub, n, m):
                """One head-contribution matmul of o-proj group (c, sub, n)."""
                if m == 0:
                    oproj_mm.po = pp.tile([128, CH], f32, tag="po", bufs=2,
                                          name="pso")
                nc.tensor.matmul(
                    oproj_mm.po[:],
                    onorm[c][m][:, bass.ts(sub, 128)],
                    wo_s[:, m * H + n * CH: m * H + (n + 1) * CH],
                    start=(m == 0), stop=(m == HQ - 1),
                )
                if m == HQ - 1:
                    o_s = wp.tile([128, CH], bf16, tag="osb", bufs=6,
                                  name="osb")
                    nc.scalar.copy(o_s[:], oproj_mm.po[:])
                    nc.sync.dma_start(
                        outD[bass.ts(c * 4 + sub, 128), bass.ts(n, CH)],
                        o_s[:],
                    )

            def attn_group(c, mi):
                pv = pp.tile([128, CH], f32, tag="acc", bufs=2, name="pspv")
                scs = [None] * SJ
                exs = [None] * SJ
                acc = None
                for j in range(SJ):
                    sc = pp.tile([128, CH], f32, tag="mm", bufs=2, name="pssc")
                    scs[j] = sc
                    nc.tensor.matmul(
                        sc[:], kt[:, bass.ts(j, 128)], qts[mi][c][:],
                        start=True, stop=True,
                    )
                    ex = wp.tile([128, CH], bf16, tag="expt", bufs=8,
                                 name="ex")
                    exs[j] = ex
                    nc.scalar.activation(ex[:], sc[:], FP.Exp, scale=SCALE)
                    if j == 0:
                        acc = ex
                    else:
                        nacc = wp.tile([128, CH], bf16, tag="exacc", bufs=2,
                                       name="exacc")
                        nc.vector.tensor_add(nacc[:], acc[:], ex[:])
                        acc = nacc
                    # filler: q-projection of next chunk, same head slot.
                    # k=15 (which triggers the rope) is deferred below so the
                    # DVE stream runs recip/onorm BEFORE the 4us rope burst,
                    # releasing this group's PSUM banks promptly.
                    if c < NCH - 1 and j < SJ - 1:
                        qproj_mm(c + 1, mi, j, wp)
                    # pv lags sc by one iteration (hides exp latency)
                    if j >= 1:
                        nc.tensor.matmul(
                            pv[:], vts[j - 1][:], exs[j - 1][:],
                            start=(j == 1), stop=False,
                        )
                    # filler: o-projection of previous chunk
                    if c > 0:
                        oproj_mm(c - 1, mi, j // 4, j % 4)
                nc.tensor.matmul(pv[:], vts[SJ - 1][:], exs[SJ - 1][:],
                                 start=False, stop=True)
                rsb = pp.tile([128, CH], f32, tag="mm", bufs=2, name="psrs")
                nc.tensor.matmul(rsb[:], ones_b[:], acc[:], start=True,
                                 stop=True)
                recipb = wp.tile([128, CH], f32, tag="recipb", bufs=2,
                                 name="rc")
                with nc.allow_low_precision(reason="1/rowsum feeds bf16 mul"):
                    nc.vector.reciprocal(recipb[:], rsb[:])
                nc.vector.tensor_tensor(onorm[c][mi][:], pv[:], recipb[:],
                                        op=ALU.mult)
                if c < NCH - 1:
                    qproj_mm(c + 1, mi, SJ - 1, wp)

            for c in range(NCH):
                for mi in range(HQ):
                    attn_group(c, mi)
            # tail: o-proj of the last chunk
            for sub in range(4):
                for n in range(4):
                    for m in range(HQ):
                        oproj_mm(NCH - 1, sub, n, m)


def build(reps=1):
    nc = bacc.Bacc("TRN2", target_bir_lowering=False, debug=False,
                   num_devices=NC)
    t = {
        "hT": nc.dram_tensor("hT", [H, S], bf16, kind="ExternalInput").ap(),
        "wq": nc.dram_tensor("wq", [128, KT * HQ * 128], bf16,
                             kind="ExternalInput").ap(),
        "wk": nc.dram_tensor("wk", [128, KT * 128], bf16,
                             kind="ExternalInput").ap(),
        "wv": nc.dram_tensor("wv", [128, KT * 128], bf16,
                             kind="ExternalInput").ap(),
        "wo": nc.dram_tensor("wo", [128, HQ * H], bf16,
                             kind="ExternalInput").ap(),
        "cc": nc.dram_tensor("cc", [128, S], bf16, kind="ExternalInput").ap(),
        "ss": nc.dram_tensor("ss", [128, S], bf16, kind="ExternalInput").ap(),
        "out": nc.dram_tensor("out", [S, H], bf16, kind="ExternalOutput").ap(),
    }
    with tile.TileContext(nc) as tc:
        for _ in range(reps):
            _body(nc, tc, t)
    nc.compile()
    return nc


# per-head rope permutation: [even dims, odd dims]
_RP = np.r_[np.arange(0, HD, 2), np.arange(1, HD, 2)]


def _bf(x):
    return np.ascontiguousarray(x).astype(ml_dtypes.bfloat16)


def prep_inputs(hidden_states, freqs_cos, freqs_sin, Wq, Wk, Wv, Wo):
    """Host-side layout prep -> list of 8 per-core input maps."""
    cosT = np.concatenate([freqs_cos.T, freqs_cos.T], 0)  # [128, S]
    sinT = np.concatenate([freqs_sin.T, freqs_sin.T], 0)
    cc = _bf(cosT)
    ss = _bf(sinT)
    wqT = Wq.T    # [H, H]
    wkT = Wk.T    # [H, 512]
    wvT = Wv.T
    woT = Wo.T    # [H, H]
    hT = [_bf(hidden_states[b].T) for b in range(B)]
    in_maps = []
    for c in range(NC):
        b, g = divmod(c, HQ)
        # wq: [p, k, m, j] -> [128, KT*HQ*128]; head cols rope-permuted
        cols = np.concatenate(
            [(4 * g + m) * HD + _RP for m in range(HQ)])      # [512]
        wq_p = wqT[:, cols].reshape(KT, 128, HQ * 128).transpose(1, 0, 2)
        wk_p = wkT[:, g * HD + _RP].reshape(KT, 128, 128).transpose(1, 0, 2)
        wv_p = wvT[:, g * HD: (g + 1) * HD].reshape(
            KT, 128, 128).transpose(1, 0, 2)
        wo_p = woT[4 * g * HD: (4 * g + 4) * HD, :].reshape(
            HQ, 128, H).transpose(1, 0, 2)
        in_maps.append({
            "hT": hT[b],
            "wq": _bf(wq_p.reshape(128, -1)),
            "wk": _bf(wk_p.reshape(128, -1)),
            "wv": _bf(wv_p.reshape(128, -1)),
            "wo": _bf(wo_p.reshape(128, -1)),
            "cc": cc, "ss": ss,
        })
    return in_maps


_CACHE = {}


def _get_nc(reps=1):
    if reps not in _CACHE:
        _CACHE[reps] = build(reps)
    return _CACHE[reps]


def kernel(hidden_states, freqs_cos, freqs_sin, Wq, Wk, Wv, Wo):
    in_maps = prep_inputs(
        np.asarray(hidden_states, np.float32),
        np.asarray(freqs_cos, np.float32),
        np.asarray(freqs_sin, np.float32),
        np.asarray(Wq, np.float32),
        np.asarray(Wk, np.float32),
        np.asarray(Wv, np.float32),
        np.asarray(Wo, np.float32),
    )
    nc = _get_nc(int(os.environ.get("KERNEL_REPS", "1")))
    res = run_bass_kernel_spmd(nc, in_maps, core_ids=list(range(NC)))
    out = np.zeros((B, S, H), np.float32)
    for c in range(NC):
        b = c // HQ
        out[b] += np.asarray(res.results[c]["out"], np.float32)
    return out


# revision 11
# speedup vs baseline: 1.4664x; 1.0431x over previous
"""Trainium2 Bass kernel for nn_CustomAttentionLayer (GQA attention + RoPE + o_proj).

Sharding: head-parallel, 8-way over (batch, kv-head-group): core c handles
batch c//4 and kv head g=c%4, i.e. query heads 4g..4g+3. Each core computes
q/k/v projections for exactly its heads (zero redundant compute), attention
over the full 2048x2048 score matrix for its 4 query heads, and a PARTIAL
output projection (contribution of its heads to the full [S,H] output).
The host sums the 4 partials per batch — no on-device collectives.

All matmul operands are bfloat16 (fp32 accumulate in PSUM): same TensorE
throughput as float32r in this regime but no small-free-dim penalty, half
the DMA traffic, and 2x DVE throughput for the softmax-denominator
accumulation. Measured end-to-end rel err ~8e-3 (budget 2e-2).

Schedule: the attention j-loop is software-pipelined (pv lags sc by one
iteration to hide the exp latency). The q-projection of the next chunk and
the o-projection of the previous chunk are interleaved into the j-loop as
TensorE filler at iters 2..14/15, with the first two matmuls of each filler
stream spilled into the previous group's tail so the group boundary (rsb ->
reciprocal -> PSUM-free chain) is covered with ready PE work. o_proj SBUF
copies are emitted ~3 iterations after their PSUM group closes so the
Activation stream (which must sustain one 570ns exp per iteration) never
head-of-line blocks on a PE matmul. Row sums come from a running DVE bf16
accumulation (2x rate) plus ONE ones-matmul per (chunk, head).
"""

import os
import numpy as np
import ml_dtypes

import concourse.bass as bass
import concourse.mybir as mybir
import concourse.tile as tile
from concourse import bacc
from concourse.bass_utils import run_bass_kernel_spmd

B, S, H = 2, 2048, 2048
NH, NKV, HD = 16, 4, 128
HQ = NH // NKV                # 4 query heads per core
NC = 8                        # cores
KT = H // 128                 # 16 contraction tiles over H
CH = 512                      # query-chunk width (PSUM bank limit)
NCH = S // CH                 # 4 chunks
SJ = S // 128                 # 16 key-position tiles
SCALE = 1.0 / float(np.sqrt(HD))

f32 = mybir.dt.float32
bf16 = mybir.dt.bfloat16
FP = mybir.ActivationFunctionType
ALU = mybir.AluOpType


def _body(nc, tc, t):
    hT, wq, wk, wv, wo, ccD, ssD, outD = (
        t["hT"], t["wq"], t["wk"], t["wv"], t["wo"], t["cc"], t["ss"], t["out"]
    )
    with tc.tile_pool(name="main", bufs=1) as main, tc.tile_pool(
        name="psum", bufs=1, space="PSUM"
    ) as pp:
        # ---------------- persistent SBUF tensors ---------------------------
        cc = main.tile([128, S], bf16, tag="cc", bufs=1)
        ss = main.tile([128, S], bf16, tag="ss", bufs=1)
        wk_s = main.tile([128, KT * 128], bf16, tag="wk", bufs=1)
        wv_s = main.tile([128, KT * 128], bf16, tag="wv", bufs=1)
        wq_s = main.tile([128, KT * HQ * 128], bf16, tag="wq", bufs=1)
        wo_s = main.tile([128, HQ * H], bf16, tag="wo", bufs=1)
        qts = [[main.tile([128, CH], bf16, tag="qt", bufs=HQ * NCH,
                          name=f"qt{m}_{c}") for c in range(NCH)]
               for m in range(HQ)]
        kt = main.tile([128, S], bf16, tag="kt", bufs=1)
        vts = [main.tile([128, HD], bf16, tag="v", bufs=SJ, name=f"v{i}")
               for i in range(SJ)]
        # h chunk c as one tile [128, k, 512]; 4 quarter-DMAs per chunk keep
        # HWDGE setup cost (665ns each) off the startup critical path.
        hc = [main.tile([128, KT * CH], bf16, tag="hc", bufs=NCH,
                        name=f"hc{c}") for c in range(NCH)]
        hT_r = hT.rearrange("(k p) n -> p k n", p=128)

        def hq_t(c, k):
            return hc[c][:, k * CH:(k + 1) * CH]

        def load_hc(c):
            for qtr in range(4):
                nc.sync.dma_start(
                    hc[c][:, qtr * 4 * CH:(qtr + 1) * 4 * CH].rearrange(
                        "p (k j) -> p k j", j=CH),
                    hT_r[:, qtr * 4:(qtr + 1) * 4, bass.ts(c, CH)],
                )

        # DMA issue order ~ consumption order so startup is never DMA-paced.
        nc.sync.dma_start(wk_s[:], wk)
        load_hc(0)
        nc.sync.dma_start(wv_s[:], wv)
        nc.sync.dma_start(cc[:], ccD)
        nc.sync.dma_start(ss[:], ssD)
        load_hc(1)
        load_hc(2)
        load_hc(3)
        for part in range(4):
            w = KT * HQ * 128 // 4
            nc.sync.dma_start(wq_s[:, part * w:(part + 1) * w],
                              wq[:, part * w:(part + 1) * w])
        nc.sync.dma_start(wo_s[:], wo)

        def rope(dst, ps, cols, pool):
            # x = [xr; xi] on partition halves, cc = [c; c], ss = [s; s]:
            #   dst[0:64]   = xr*c - xi*s
            #   dst[64:128] = xr*s + xi*c
            w = dst.shape[-1]
            t1 = pool.tile([64, w], f32, tag="ropeA", bufs=2, name="t1")
            t2 = pool.tile([64, w], f32, tag="ropeB", bufs=2, name="t2")
            nc.vector.tensor_tensor(t1[:], ps[64:128, :], ss[64:128, cols],
                                    op=ALU.mult)
            nc.vector.tensor_tensor(t2[:], ps[0:64, :], cc[0:64, cols],
                                    op=ALU.mult)
            nc.vector.tensor_sub(dst[0:64, :], t2[:], t1[:])
            nc.vector.tensor_tensor(t1[:], ps[0:64, :], ss[0:64, cols],
                                    op=ALU.mult)
            nc.vector.tensor_tensor(t2[:], ps[64:128, :], cc[64:128, cols],
                                    op=ALU.mult)
            nc.vector.tensor_add(dst[64:128, :], t2[:], t1[:])

        def qproj_mm(c, m, k, wp):
            """One contraction-tile matmul of q-projection (c, m)."""
            if k == 0:
                qproj_mm.ps = pp.tile([128, CH], f32, tag="psq", bufs=2,
                                      name="psq")
            nc.tensor.matmul(
                qproj_mm.ps[:],
                wq_s[:, k * HQ * 128 + m * 128: k * HQ * 128 + (m + 1) * 128],
                hq_t(c, k),
                start=(k == 0), stop=(k == KT - 1),
            )
            if k == KT - 1:
                rope(qts[m][c][:], qproj_mm.ps, bass.ts(c, CH), wp)

        with tc.tile_pool(name="work", bufs=1) as wp:
            # ------- k+v projections, interleaved per chunk -----------------
            for c in range(NCH):
                cols = bass.ts(c, CH)
                ps = pp.tile([128, CH], f32, tag="mm", bufs=2, name="psk")
                for k in range(KT):
                    nc.tensor.matmul(
                        ps[:], wk_s[:, bass.ts(k, 128)], hq_t(c, k),
                        start=(k == 0), stop=(k == KT - 1),
                    )
                rope(kt[:, cols], ps, cols, wp)
                pv4 = pp.tile([128, 4 * HD], f32, tag="po", bufs=2, name="psv")
                for sub in range(4):
                    for k in range(KT):
                        nc.tensor.matmul(
                            pv4[:, bass.ts(sub, HD)],
                            hc[c][:, k * CH + sub * 128: k * CH + (sub + 1) * 128],
                            wv_s[:, bass.ts(k, 128)],
                            start=(k == 0), stop=(k == KT - 1),
                        )
                for sub in range(4):
                    nc.scalar.copy(vts[c * 4 + sub][:], pv4[:, bass.ts(sub, HD)])

            # ---------------- q projection chunk 0 --------------------------
            for m in range(HQ):
                for k in range(KT):
                    qproj_mm(0, m, k, wp)

            # ---------------- attention + interleaved q/o-proj --------------
            ones_b = wp.tile([128, 128], bf16, tag="ones", bufs=1)
            nc.vector.memset(ones_b[:], 1.0)

            onorm = [[wp.tile([128, CH], bf16, tag="onorm", bufs=2 * HQ,
                              name=f"on{c}_{m}") for m in range(HQ)]
                     for c in range(NCH)]

            o_pend = {}   # n -> psum tile awaiting copy+DMA

            def oproj_mm(c, sub, i):
                """o-matmul #i (n=i//4, m=i%4) of subgroup (c, sub)."""
                n, m = divmod(i, 4)
                if m == 0:
                    o_pend[n] = pp.tile([128, CH], f32, tag="po", bufs=2,
                                        name="pso")
                nc.tensor.matmul(
                    o_pend[n][:],
                    onorm[c][m][:, bass.ts(sub, 128)],
                    wo_s[:, m * H + n * CH: m * H + (n + 1) * CH],
                    start=(m == 0), stop=(m == HQ - 1),
                )

            def oproj_copy(c, sub, n):
                o_s = wp.tile([128, CH], bf16, tag="osb", bufs=6, name="osb")
                nc.scalar.copy(o_s[:], o_pend.pop(n)[:])
                nc.sync.dma_start(
                    outD[bass.ts(c * 4 + sub, 128), bass.ts(n, CH)], o_s[:])

            # group g = c*4 + mi. Filler streams per group:
            #   qfill(g): q-proj of (c+1, mi)      (exists iff c < NCH-1)
            #   ofill(g): o-proj subgroup (c-1, mi) (exists iff c > 0)
            # with each stream's first two matmuls emitted at the previous
            # group's tail, and qfill's k=15 (+rope) at its own tail.
            def qfill_of(g):
                c, mi = divmod(g, HQ)
                return (c + 1, mi) if c < NCH - 1 else None

            def ofill_of(g):
                if g is None:
                    return None
                c, mi = divmod(g, HQ)
                if g <= NCH * HQ:
                    c, mi = divmod(g, HQ)
                    if c >= 1:
                        return (c - 1, mi)
                return None

            def osub_of(g):
                # linear o-subgroup index: groups 4..19 map to (c-1, sub)
                if g < HQ or g >= 5 * HQ:
                    return None
                return divmod(g, HQ)[0] - 1, divmod(g, HQ)[1]

            def group_tail(g):
                """Emit boundary spill-over: qfill k15+rope of group g, o#0/#1
                of group g+1's subgroup, q k0/k1 of group g+1's qfill, and the
                delayed copy of subgroup(g)'s last po group."""
                qf = qfill_of(g) if g >= 0 else None
                if qf is not None:
                    qproj_mm(qf[0], qf[1], SJ - 1, wp)
                osub = osub_of(g) if g >= 0 else None
                if osub is not None:
                    oproj_copy(osub[0], osub[1], 3)
                nosub = osub_of(g + 1)
                if nosub is not None:
                    oproj_mm(nosub[0], nosub[1], 0)
                    oproj_mm(nosub[0], nosub[1], 1)
                nqf = qfill_of(g + 1) if g + 1 < NCH * HQ else None
                if nqf is not None:
                    qproj_mm(nqf[0], nqf[1], 0, wp)
                    qproj_mm(nqf[0], nqf[1], 1, wp)

            def attn_group(g):
                c, mi = divmod(g, HQ)
                qf = qfill_of(g)
                osub = osub_of(g)
                pv = pp.tile([128, CH], f32, tag="acc", bufs=2, name="pspv")
                exs = [None] * SJ
                acc = None
                for j in range(SJ):
                    sc = pp.tile([128, CH], f32, tag="mm", bufs=2, name="pssc")
                    nc.tensor.matmul(
                        sc[:], kt[:, bass.ts(j, 128)], qts[mi][c][:],
                        start=True, stop=True,
                    )
                    ex = wp.tile([128, CH], bf16, tag="expt", bufs=8,
                                 name="ex")
                    exs[j] = ex
                    nc.scalar.activation(ex[:], sc[:], FP.Exp, scale=SCALE)
                    if j == 0:
                        acc = ex
                    else:
                        nacc = wp.tile([128, CH], bf16, tag="exacc", bufs=2,
                                       name="exacc")
                        nc.vector.tensor_add(nacc[:], acc[:], ex[:])
                        acc = nacc
                    if qf is not None and 2 <= j <= SJ - 2:
                        qproj_mm(qf[0], qf[1], j, wp)
                    if j >= 1:
                        nc.tensor.matmul(
                            pv[:], vts[j - 1][:], exs[j - 1][:],
                            start=(j == 1), stop=False,
                        )
                    if osub is not None and j >= 2:
                        oproj_mm(osub[0], osub[1], j)
                        if j in (6, 10, 14):
                            oproj_copy(osub[0], osub[1], (j - 6) // 4)
                nc.tensor.matmul(pv[:], vts[SJ - 1][:], exs[SJ - 1][:],
                                 start=False, stop=True)
                rsb = pp.tile([128, CH], f32, tag="mm", bufs=2, name="psrs")
                nc.tensor.matmul(rsb[:], ones_b[:], acc[:], start=True,
                                 stop=True)
                group_tail(g)
                recipb = wp.tile([128, CH], f32, tag="recipb", bufs=2,
                                 name="rc")
                with nc.allow_low_precision(reason="1/rowsum feeds bf16 mul"):
                    nc.vector.reciprocal(recipb[:], rsb[:])
                nc.vector.tensor_tensor(onorm[c][mi][:], pv[:], recipb[:],
                                        op=ALU.mult)

            # startup counterpart of group_tail(-1): q k0/k1 of group 0's
            # qfill stream
            qproj_mm(1, 0, 0, wp)
            qproj_mm(1, 0, 1, wp)

            for g in range(NCH * HQ):
                attn_group(g)

            # ---------------- tail: o-proj of the last chunk ----------------
            for sub in range(HQ):
                first = 2 if sub == 0 else 0   # (c3, 0) #0/#1 spilled above
                for i in range(first, 16):
                    oproj_mm(NCH - 1, sub, i)
                    if i % 4 == 3:
                        oproj_copy(NCH - 1, sub, i // 4)


def build(reps=1):
    nc = bacc.Bacc("TRN2", target_bir_lowering=False, debug=False,
                   num_devices=NC)
    t = {
        "hT": nc.dram_tensor("hT", [H, S], bf16, kind="ExternalInput").ap(),
        "wq": nc.dram_tensor("wq", [128, KT * HQ * 128], bf16,
                             kind="ExternalInput").ap(),
        "wk": nc.dram_tensor("wk", [128, KT * 128], bf16,
                             kind="ExternalInput").ap(),
        "wv": nc.dram_tensor("wv", [128, KT * 128], bf16,
                             kind="ExternalInput").ap(),
        "wo": nc.dram_tensor("wo", [128, HQ * H], bf16,
                             kind="ExternalInput").ap(),
        "cc": nc.dram_tensor("cc", [128, S], bf16, kind="ExternalInput").ap(),
        "ss": nc.dram_tensor("ss", [128, S], bf16, kind="ExternalInput").ap(),
        "out": nc.dram_tensor("out", [S, H], bf16, kind="ExternalOutput").ap(),
    }
    with tile.TileContext(nc) as tc:
        for _ in range(reps):
            _body(nc, tc, t)
    nc.compile()
    return nc


# per-head rope permutation: [even dims, odd dims]
_RP = np.r_[np.arange(0, HD, 2), np.arange(1, HD, 2)]


def _bf(x):
    return np.ascontiguousarray(x).astype(ml_dtypes.bfloat16)


def prep_inputs(hidden_states, freqs_cos, freqs_sin, Wq, Wk, Wv, Wo):
    """Host-side layout prep -> list of 8 per-core input maps."""
    cosT = np.concatenate([freqs_cos.T, freqs_cos.T], 0)  # [128, S]
    sinT = np.concatenate([freqs_sin.T, freqs_sin.T], 0)
    cc = _bf(cosT)
    ss = _bf(sinT)
    wqT = Wq.T    # [H, H]
    wkT = Wk.T    # [H, 512]
    wvT = Wv.T
    woT = Wo.T    # [H, H]
    hT = [_bf(hidden_states[b].T) for b in range(B)]
    in_maps = []
    for c in range(NC):
        b, g = divmod(c, HQ)
        # wq: [p, k, m, j] -> [128, KT*HQ*128]; head cols rope-permuted
        cols = np.concatenate(
            [(4 * g + m) * HD + _RP for m in range(HQ)])      # [512]
        wq_p = wqT[:, cols].reshape(KT, 128, HQ * 128).transpose(1, 0, 2)
        wk_p = wkT[:, g * HD + _RP].reshape(KT, 128, 128).transpose(1, 0, 2)
        wv_p = wvT[:, g * HD: (g + 1) * HD].reshape(
            KT, 128, 128).transpose(1, 0, 2)
        wo_p = woT[4 * g * HD: (4 * g + 4) * HD, :].reshape(
            HQ, 128, H).transpose(1, 0, 2)
        in_maps.append({
            "hT": hT[b],
            "wq": _bf(wq_p.reshape(128, -1)),
            "wk": _bf(wk_p.reshape(128, -1)),
            "wv": _bf(wv_p.reshape(128, -1)),
            "wo": _bf(wo_p.reshape(128, -1)),
            "cc": cc, "ss": ss,
        })
    return in_maps


_CACHE = {}


def _get_nc(reps=1):
    if reps not in _CACHE:
        _CACHE[reps] = build(reps)
    return _CACHE[reps]


def kernel(hidden_states, freqs_cos, freqs_sin, Wq, Wk, Wv, Wo):
    in_maps = prep_inputs(
        np.asarray(hidden_states, np.float32),
        np.asarray(freqs_cos, np.float32),
        np.asarray(freqs_sin, np.float32),
        np.asarray(Wq, np.float32),
        np.asarray(Wk, np.float32),
        np.asarray(Wv, np.float32),
        np.asarray(Wo, np.float32),
    )
    nc = _get_nc(int(os.environ.get("KERNEL_REPS", "1")))
    res = run_bass_kernel_spmd(nc, in_maps, core_ids=list(range(NC)))
    out = np.zeros((B, S, H), np.float32)
    for c in range(NC):
        b = c // HQ
        out[b] += np.asarray(res.results[c]["out"], np.float32)
    return out
